# revision 3
# baseline (speedup 1.0000x reference)
"""CenterLoss Trainium2 kernel.

loss = mean_i clip(||features_i - centers[target_i]||^2, 1e-12, 1e12)
       + (NUM_CLASSES-1) * 1e-12        # the clipped zeros of the masked distmat

The reference builds the full [8192, 2048] distance matrix and masks out
everything but the target column; only the per-row target distance matters,
so the kernel is a gather + fused (f-c)^2-reduce:

  - data-parallel over the batch: 1024 rows per core on 8 cores
  - centers stay in HBM; rows are gathered on-device with dma_gather
    (SWDGE extended instruction) keyed by int16 target indices
  - DVE computes diff = f - c and a fused square+per-partition-accumulate
  - per-chunk partials land in a [128, NCH] tile; the 8 per-core partial
    tiles are summed on the host (the "all-reduce" of the scalar loss)

dma_gather writes gathered row i to partition i % 128, free slot i // 128,
so the host lays features out the same way: batch row r of a core's shard
lives at partition r % 128, slot r // 128. Indices are wrapped int16
[16, num_idxs/16] with element [p, s] = idx[s*16 + p], replicated to all
128 partitions.
"""

import numpy as np

import concourse.bacc as bacc
import concourse.bass as bass
import concourse.tile as tile
from concourse import mybir
from concourse.bass_utils import run_bass_kernel_spmd

N_CORES = 8
BATCH = 8192
FEAT = 512
NCLS = 2048
P = 128

ROWS = BATCH // N_CORES          # 1024 rows per core
SLOTS = ROWS // P                # 8 gathered rows per partition
FREE = SLOTS * FEAT              # 4096 f32 per partition
NSPLIT = 2                       # dma_gather calls per core
G_IDX = ROWS // NSPLIT           # 512 indices per gather call
G_SLOTS = SLOTS // NSPLIT        # 4 slots per gather call
NCH = 4                          # compute chunks per core
CH_SLOTS = SLOTS // NCH          # 2 slots per compute chunk
CH_FREE = CH_SLOTS * FEAT        # 1024 f32 per partition per chunk
IDX_COLS = ROWS // 16            # 64 wrapped int16 index columns

_CACHE: dict[str, object] = {}


def _build_nc():
    nc = bacc.Bacc("TRN2", target_bir_lowering=False, debug=False)

    feats = nc.dram_tensor("features", [P, FREE], mybir.dt.float32, kind="ExternalInput")
    centers = nc.dram_tensor("centers", [NCLS, FEAT], mybir.dt.float32, kind="ExternalInput")
    idxs = nc.dram_tensor("idxs", [P, IDX_COLS], mybir.dt.int16, kind="ExternalInput")
    partials = nc.dram_tensor("partials", [P, NCH], mybir.dt.float32, kind="ExternalOutput")

    with tile.TileContext(nc) as tc:
        with (
            tc.tile_pool(name="idxp", bufs=1) as idxp,
            tc.tile_pool(name="accp", bufs=1) as accp,
            tc.tile_pool(name="cgp", bufs=1) as cgp,
            tc.tile_pool(name="fp", bufs=2) as fp,
            tc.tile_pool(name="dp", bufs=2) as dp,
            tc.tile_pool(name="sp", bufs=2) as sp,
        ):
            idx_t = idxp.tile([P, IDX_COLS], mybir.dt.int16)
            nc.sync.dma_start(idx_t[:], idxs[:])

            accs = accp.tile([P, NCH], mybir.dt.float32)
            c_t = cgp.tile([P, SLOTS, FEAT], mybir.dt.float32)

            for j in range(NSPLIT):
                cols = IDX_COLS // NSPLIT
                nc.gpsimd.dma_gather(
                    c_t[:, j * G_SLOTS:(j + 1) * G_SLOTS, :],
                    centers[:],
                    idx_t[:, j * cols:(j + 1) * cols],
                    G_IDX,
                    G_IDX,
                    FEAT,
                )

            for k in range(NCH):
                f_t = fp.tile([P, CH_FREE], mybir.dt.float32)
                nc.sync.dma_start(f_t[:], feats[:, k * CH_FREE:(k + 1) * CH_FREE])

                d_t = dp.tile([P, CH_FREE], mybir.dt.float32)
                nc.vector.tensor_tensor(
                    out=d_t[:],
                    in0=f_t[:],
                    in1=c_t[:, k * CH_SLOTS:(k + 1) * CH_SLOTS, :].rearrange(
                        "p a b -> p (a b)"
                    ),
                    op=mybir.AluOpType.subtract,
                )

                # out = (d * 1.0) * d = d^2, accum_out = sum(out) per partition.
                # DVE's exact f32 ALU (ACT Square is LUT-based).
                s_t = sp.tile([P, CH_FREE], mybir.dt.float32)
                nc.vector.scalar_tensor_tensor(
                    out=s_t[:],
                    in0=d_t[:],
                    scalar=1.0,
                    in1=d_t[:],
                    op0=mybir.AluOpType.mult,
                    op1=mybir.AluOpType.mult,
                    accum_out=accs[:, k:k + 1],
                )

            nc.sync.dma_start(partials[:], accs[:])

    nc.compile()
    return nc


def _get_nc():
    if "nc" not in _CACHE:
        _CACHE["nc"] = _build_nc()
    return _CACHE["nc"]


def _prep_inputs(features: np.ndarray, centers: np.ndarray, target: np.ndarray):
    """Shard + relayout host-side. Row r of core i's shard (global row
    1024*i + r) goes to partition r % 128, slot r // 128."""
    feats = (
        np.ascontiguousarray(features, dtype=np.float32)
        .reshape(N_CORES, SLOTS, P, FEAT)
        .transpose(0, 2, 1, 3)
        .reshape(N_CORES, P, FREE)
    )
    feats = np.ascontiguousarray(feats)
    cent = np.ascontiguousarray(centers, dtype=np.float32)
    t = target.astype(np.int16).reshape(N_CORES, IDX_COLS, 16)
    wrapped = t.transpose(0, 2, 1)  # [cores, 16, IDX_COLS]: [p, s] = t[s*16+p]
    idx = np.ascontiguousarray(np.tile(wrapped, (1, P // 16, 1)))  # [cores, 128, IDX_COLS]
    return feats, cent, idx


def kernel(features: np.ndarray, centers: np.ndarray, target: np.ndarray) -> np.ndarray:
    nc = _get_nc()
    feats, cent, idx = _prep_inputs(features, centers, target)

    in_maps = [
        {"features": feats[i], "centers": cent, "idxs": idx[i]}
        for i in range(N_CORES)
    ]
    res = run_bass_kernel_spmd(nc, in_maps, core_ids=list(range(N_CORES)))

    total = 0.0
    for r in res.results:
        total += float(r["partials"].astype(np.float64).sum())
    loss = total / BATCH + (NCLS - 1) * 1e-12
    return np.asarray(loss, dtype=np.float32)


# revision 5
# speedup vs baseline: 1.0479x; 1.0479x over previous
"""CenterLoss Trainium2 kernel.

loss = mean_i clip(||features_i - centers[target_i]||^2, 1e-12, 1e12)
       + (NUM_CLASSES-1) * 1e-12        # the clipped zeros of the masked distmat

The reference builds the full [8192, 2048] distance matrix and masks out
everything but the target column; only the per-row target distance matters,
so the kernel is a gather + fused (f-c)^2-reduce:

  - data-parallel over the batch: 1024 rows per core on 8 cores
  - centers stay in HBM; rows are gathered on-device with dma_gather
    (SWDGE extended instruction) keyed by int16 target indices
  - DVE computes diff = f - c and a fused square+per-partition-accumulate
  - per-chunk partials land in a [128, NCH] tile; the 8 per-core partial
    tiles are summed on the host (the "all-reduce" of the scalar loss)

dma_gather writes gathered row i to partition i % 128, free slot i // 128,
so the host lays features out the same way: batch row r of a core's shard
lives at partition r % 128, slot r // 128. Indices are wrapped int16
[16, num_idxs/16] with element [p, s] = idx[s*16 + p], replicated to all
128 partitions.
"""

import numpy as np

import concourse.bacc as bacc
import concourse.bass as bass
import concourse.tile as tile
from concourse import library_config, mybir
from concourse.bass_utils import run_bass_kernel_spmd

N_CORES = 8
BATCH = 8192
FEAT = 512
NCLS = 2048
P = 128

ROWS = BATCH // N_CORES          # 1024 rows per core
SLOTS = ROWS // P                # 8 gathered rows per partition
FREE = SLOTS * FEAT              # 4096 f32 per partition
NSPLIT = 2                       # dma_gather calls per core
G_IDX = ROWS // NSPLIT           # 512 indices per gather call
G_SLOTS = SLOTS // NSPLIT        # 4 slots per gather call
NCH = 4                          # compute chunks per core
CH_SLOTS = SLOTS // NCH          # 2 slots per compute chunk
CH_FREE = CH_SLOTS * FEAT        # 1024 f32 per partition per chunk
IDX_COLS = ROWS // 16            # 64 wrapped int16 index columns

_CACHE: dict[str, object] = {}


def _build_nc():
    nc = bacc.Bacc("TRN2", target_bir_lowering=False, debug=False)

    feats = nc.dram_tensor("features", [P, FREE], mybir.dt.float32, kind="ExternalInput")
    centers = nc.dram_tensor("centers", [NCLS, FEAT], mybir.dt.float32, kind="ExternalInput")
    idxs = nc.dram_tensor("idxs", [P, IDX_COLS], mybir.dt.int16, kind="ExternalInput")
    partials = nc.dram_tensor("partials", [P, NCH], mybir.dt.float32, kind="ExternalOutput")

    with tile.TileContext(nc) as tc:
        with (
            tc.tile_pool(name="idxp", bufs=1) as idxp,
            tc.tile_pool(name="accp", bufs=1) as accp,
            tc.tile_pool(name="cgp", bufs=1) as cgp,
            tc.tile_pool(name="fp", bufs=4) as fp,
            tc.tile_pool(name="dp", bufs=2) as dp,
            tc.tile_pool(name="sp", bufs=2) as sp,
        ):
            # hoist the Q7 library switch ahead of everything so its drain
            # hides under the kernel-entry barrier instead of gating gathers
            nc.gpsimd.load_library(library_config.mlp)

            # idx load on the ACT HWDGE ring — doesn't queue behind the
            # feature loads on the SP ring, and gates the gathers
            idx_t = idxp.tile([P, IDX_COLS], mybir.dt.int16)
            nc.scalar.dma_start(idx_t[:], idxs[:])

            accs = accp.tile([P, NCH], mybir.dt.float32)
            c_t = cgp.tile([P, SLOTS, FEAT], mybir.dt.float32)

            for j in range(NSPLIT):
                cols = IDX_COLS // NSPLIT
                nc.gpsimd.dma_gather(
                    c_t[:, j * G_SLOTS:(j + 1) * G_SLOTS, :],
                    centers[:],
                    idx_t[:, j * cols:(j + 1) * cols],
                    G_IDX,
                    G_IDX,
                    FEAT,
                )

            for k in range(NCH):
                f_t = fp.tile([P, CH_FREE], mybir.dt.float32)
                nc.sync.dma_start(f_t[:], feats[:, k * CH_FREE:(k + 1) * CH_FREE])

                d_t = dp.tile([P, CH_FREE], mybir.dt.float32)
                nc.vector.tensor_tensor(
                    out=d_t[:],
                    in0=f_t[:],
                    in1=c_t[:, k * CH_SLOTS:(k + 1) * CH_SLOTS, :].rearrange(
                        "p a b -> p (a b)"
                    ),
                    op=mybir.AluOpType.subtract,
                )

                # Square + per-partition accumulate on ACT: measured bit-exact
                # (elementwise == f32 multiply, accum == f32 sequential sum),
                # and it frees DVE to pipeline the subtracts.
                s_t = sp.tile([P, CH_FREE], mybir.dt.float32)
                nc.scalar.activation(
                    out=s_t[:],
                    in_=d_t[:],
                    func=mybir.ActivationFunctionType.Square,
                    accum_out=accs[:, k:k + 1],
                )

            nc.sync.dma_start(partials[:], accs[:])

    nc.compile()
    return nc


def _get_nc():
    if "nc" not in _CACHE:
        _CACHE["nc"] = _build_nc()
    return _CACHE["nc"]


def _prep_inputs(features: np.ndarray, centers: np.ndarray, target: np.ndarray):
    """Shard + relayout host-side. Row r of core i's shard (global row
    1024*i + r) goes to partition r % 128, slot r // 128."""
    feats = (
        np.ascontiguousarray(features, dtype=np.float32)
        .reshape(N_CORES, SLOTS, P, FEAT)
        .transpose(0, 2, 1, 3)
        .reshape(N_CORES, P, FREE)
    )
    feats = np.ascontiguousarray(feats)
    cent = np.ascontiguousarray(centers, dtype=np.float32)
    t = target.astype(np.int16).reshape(N_CORES, IDX_COLS, 16)
    wrapped = t.transpose(0, 2, 1)  # [cores, 16, IDX_COLS]: [p, s] = t[s*16+p]
    idx = np.ascontiguousarray(np.tile(wrapped, (1, P // 16, 1)))  # [cores, 128, IDX_COLS]
    return feats, cent, idx


def kernel(features: np.ndarray, centers: np.ndarray, target: np.ndarray) -> np.ndarray:
    nc = _get_nc()
    feats, cent, idx = _prep_inputs(features, centers, target)

    in_maps = [
        {"features": feats[i], "centers": cent, "idxs": idx[i]}
        for i in range(N_CORES)
    ]
    res = run_bass_kernel_spmd(nc, in_maps, core_ids=list(range(N_CORES)))

    total = 0.0
    for r in res.results:
        total += float(r["partials"].astype(np.float64).sum())
    loss = total / BATCH + (NCLS - 1) * 1e-12
    return np.asarray(loss, dtype=np.float32)


# revision 11
# speedup vs baseline: 1.5259x; 1.4561x over previous
"""CenterLoss Trainium2 kernel (raw bacc, explicit semaphores).

loss = mean_i clip(||features_i - centers[target_i]||^2, 1e-12, 1e12)
       + (NUM_CLASSES-1) * 1e-12        # the clipped zeros of the masked distmat

The reference builds the full [8192, 2048] distance matrix and masks out
everything but the target column; only the per-row target distance matters,
so the kernel is a gather + (f-c)^2-reduce:

  - data-parallel over the batch: 1024 rows per core on 8 cores
  - centers stay in HBM; per slot g (128 rows, one per partition) an
    indirect SWDGE DMA gathers centers[idx[p, g]] -> c_t[p, g*512:...]
  - DVE computes diff = f - c per slot; ACT squares with fused
    per-partition accumulate into acc[:, g]
  - the per-core [128, 8] partial tiles are summed on the host (the
    "all-reduce" of the scalar loss)

Layout per core: shard row r (0..1023) lives at partition r // 8, slot
r % 8 (the natural contiguous [1024, 512] -> [128, 8*512] reshape);
idx[p, g] = target[8p + g].

Ordering notes (from profiling):
  - the idx load goes first and the feature loads wait for its semaphore —
    otherwise the tiny idx transfer's 16 sem increments trickle out behind
    2 MB of feature packets in the SDMA round-robin and gate the gathers
    ~10 us late
  - indirect_dma_start (InstDMACopy + dynamic AP) gathers one row per
    partition per call; per-call cost is ~1.1 us of Q7 descgen, no
    extended-instruction library load (dma_gather would stall ~6 us on
    LOAD_LIB ucode fetch)
  - ACT's Square is bit-exact for f32 (measured: elementwise == f32
    multiply, accum == f32 sequential sum)
"""

from contextlib import ExitStack

import numpy as np

import concourse.bacc as bacc
import concourse.bass as bass
from concourse import mybir
from concourse.bass_utils import run_bass_kernel_spmd

N_CORES = 8
BATCH = 8192
FEAT = 512
NCLS = 2048
P = 128

ROWS = BATCH // N_CORES          # 1024 rows per core
SLOTS = ROWS // P                # 8 rows per partition = 8 gather calls
FREE = SLOTS * FEAT              # 4096 f32 per partition
FHALF = FREE // 2                # feature DMA granularity (2 x 1 MB)

_CACHE: dict[str, object] = {}

F32 = mybir.dt.float32


def _build_nc():
    nc = bacc.Bacc("TRN2", target_bir_lowering=False, debug=False)

    feats = nc.dram_tensor("features", [P, FREE], F32, kind="ExternalInput")
    centers = nc.dram_tensor("centers", [NCLS, FEAT], F32, kind="ExternalInput")
    idxs = nc.dram_tensor("idxs", [P, SLOTS], mybir.dt.int32, kind="ExternalInput")
    partials = nc.dram_tensor("partials", [P, SLOTS], F32, kind="ExternalOutput")

    with (
        nc.sbuf_tensor("f_t", [P, FREE], F32) as f_t,
        nc.sbuf_tensor("c_t", [P, FREE], F32) as c_t,
        nc.sbuf_tensor("d_t", [P, FREE], F32) as d_t,
        nc.sbuf_tensor("idx_t", [P, SLOTS], mybir.dt.int32) as idx_t,
        nc.sbuf_tensor("acc", [P, SLOTS], F32) as acc,
        nc.semaphore("s_idx") as s_idx,
        nc.semaphore("s_f0") as s_f0,
        nc.semaphore("s_f1") as s_f1,
        nc.semaphore("s_sub") as s_sub,
        nc.semaphore("s_sq") as s_sq,
        nc.semaphore("s_out") as s_out,
        ExitStack() as stack,
    ):
        # one semaphore per gather DMA: a shared counting sem is racy —
        # per-SDMA-engine completion skew means a cumulative count can hit
        # 16*(g+1) while some engine still owes call g's last bytes
        s_gath = [
            stack.enter_context(nc.semaphore(f"s_g{g}")) for g in range(SLOTS)  # noqa: ANT232
        ]
        s_feat = [s_f0, s_f1]
        block = stack.enter_context(nc.Block())

        @block.sync
        def _(sync: bass.BassEngine):
            # idx first and alone: its completion gates all gathers
            sync.dma_start(idx_t[:], idxs[:]).then_inc(s_idx, 16)
            sync.wait_ge(s_idx, 16)
            for h in range(2):
                sync.dma_start(
                    f_t[:, h * FHALF:(h + 1) * FHALF],
                    feats[:, h * FHALF:(h + 1) * FHALF],
                ).then_inc(s_feat[h], 16)
            sync.wait_ge(s_sq, SLOTS)
            sync.dma_start(partials[:], acc[:]).then_inc(s_out, 16)
            sync.wait_ge(s_out, 16)

        @block.gpsimd
        def _(gpsimd: bass.BassGpSimd):
            gpsimd.wait_ge(s_idx, 16)
            for g in range(SLOTS):
                gpsimd.indirect_dma_start(
                    out=c_t[:, g * FEAT:(g + 1) * FEAT],
                    out_offset=None,
                    in_=centers[:],
                    in_offset=bass.IndirectOffsetOnAxis(
                        ap=idx_t[:, g:g + 1], axis=0
                    ),
                ).then_inc(s_gath[g], 16)

        @block.vector
        def _(vector: bass.BassEngine):
            for g in range(SLOTS):
                vector.wait_ge(s_gath[g], 16)
                vector.wait_ge(s_feat[g // (SLOTS // 2)], 16)
                vector.tensor_tensor(
                    out=d_t[:, g * FEAT:(g + 1) * FEAT],
                    in0=f_t[:, g * FEAT:(g + 1) * FEAT],
                    in1=c_t[:, g * FEAT:(g + 1) * FEAT],
                    op=mybir.AluOpType.subtract,
                ).then_inc(s_sub, 1)

        @block.scalar
        def _(scalar: bass.BassEngine):
            for g in range(SLOTS):
                scalar.wait_ge(s_sub, g + 1)
                # in-place square: ACT streams read-before-write per element
                scalar.activation(
                    out=d_t[:, g * FEAT:(g + 1) * FEAT],
                    in_=d_t[:, g * FEAT:(g + 1) * FEAT],
                    func=mybir.ActivationFunctionType.Square,
                    accum_out=acc[:, g:g + 1],
                ).then_inc(s_sq, 1)

    nc.compile()
    return nc


def _get_nc():
    if "nc" not in _CACHE:
        _CACHE["nc"] = _build_nc()
    return _CACHE["nc"]


def _prep_inputs(features: np.ndarray, centers: np.ndarray, target: np.ndarray):
    """Shard host-side. Core i takes rows [1024*i, 1024*(i+1)); within a
    core, partition p holds rows 8p..8p+7 contiguously (natural reshape)."""
    feats = np.ascontiguousarray(features, dtype=np.float32).reshape(N_CORES, P, FREE)
    cent = np.ascontiguousarray(centers, dtype=np.float32)
    idx = np.ascontiguousarray(target.astype(np.int32)).reshape(N_CORES, P, SLOTS)
    return feats, cent, idx


def kernel(features: np.ndarray, centers: np.ndarray, target: np.ndarray) -> np.ndarray:
    nc = _get_nc()
    feats, cent, idx = _prep_inputs(features, centers, target)

    in_maps = [
        {"features": feats[i], "centers": cent, "idxs": idx[i]}
        for i in range(N_CORES)
    ]
    res = run_bass_kernel_spmd(nc, in_maps, core_ids=list(range(N_CORES)))

    total = 0.0
    for r in res.results:
        total += float(r["partials"].astype(np.float64).sum())
    loss = total / BATCH + (NCLS - 1) * 1e-12
    return np.asarray(loss, dtype=np.float32)
